# revision 22
# baseline (speedup 1.0000x reference)
"""Trainium2 Bass kernel for nn_Application_85469849191065 (moe_routing).

Data-parallel over the batch dim N=4096 across 8 NeuronCores (512 samples
per core); the small expert weight banks are replicated to every core.

On-device layout is feature-major (activations stored transposed,
[features, samples]) so every matmul's contraction dim lands on the
partition axis with no per-layer transposes.  The per-sample expert blend
  out_n = sum_e coeff[n,e] * (x_n @ W[e] + b[e])
is computed as scale-input MoE: the moving operand of each contraction
chunk is (coeff_e broadcast across partitions) * X^T, so the blend over
experts accumulates for free in PSUM.  The bias term rides spare zero
rows of the latent contraction chunk whose moving rows hold coeff^T.
Compute dtype is bf16 (weights cast f32->bf16 in-flight by the gpsimd
DMA engines), accumulation fp32 in PSUM.
"""

import numpy as np

import concourse.bass as bass
import concourse.bacc as bacc
import concourse.mybir as mybir
import concourse.tile as tile
from concourse.bass_utils import run_bass_kernel_spmd

F32 = mybir.dt.float32
BF16 = mybir.dt.bfloat16
AF = mybir.ActivationFunctionType
ALU = mybir.AluOpType

N = 4096
NCORES = 8
NS = N // NCORES  # 512 samples per core
LAT, COND, PH2, E = 64, 219, 40, 8
H, OUT = 512, 171
GIN = PH2 + LAT  # 104
X0R = COND + LAT  # 283

L0C = 3   # per-expert L0 chunks: cond[0:128], cond[128:219], latent(64)
L1C = 5   # 4 h chunks + the shared latent chunk
L2C = 4
NJ0 = E * L0C  # 24
NJ1 = E * L1C  # 40
NJ2 = E * L2C  # 32

_CACHE = {}


def build():
    nc = bacc.Bacc("TRN2", target_bir_lowering=False, debug=False,
                   num_devices=NCORES)

    # ---- dram parameters (per-core shard shapes) ----
    # xgx0: chunk0 = gate input (104 rows), chunks 1..3 = x0 chunks
    xgx0_ext = nc.dram_tensor("xgx0", [128, 1 + L0C, NS], F32,
                              kind="ExternalInput")
    # smalls: gw1|gw2|gw3|gb12|gb3|ident|b2  packed along free dim
    smalls_ext = nc.dram_tensor("smalls", [128, 128 + 128 + E + 2 + 1 + 128 + OUT],
                                F32, kind="ExternalInput")
    # trimmed weight groups (no zero-pad rows on the wire)
    w0a_ext = nc.dram_tensor("w0a", [128, E, H], F32, kind="ExternalInput")
    w0b_ext = nc.dram_tensor("w0b", [COND - 128, E, H], F32, kind="ExternalInput")
    w0c_ext = nc.dram_tensor("w0c", [LAT + E, E, H], F32, kind="ExternalInput")
    w1l_ext = nc.dram_tensor("w1l", [LAT + E, E, H], F32, kind="ExternalInput")
    w1a_ext = nc.dram_tensor("w1a", [128, 4, 4, H], F32, kind="ExternalInput")
    w1b_ext = nc.dram_tensor("w1b", [128, 4, 4, H], F32, kind="ExternalInput")
    w2_ext = nc.dram_tensor("w2", [128, NJ2, OUT], F32, kind="ExternalInput")
    predt_ext = nc.dram_tensor("predt", [OUT, NS], F32, kind="ExternalOutput")
    coeff4_ext = nc.dram_tensor("coeff4", [128, NS // 128, E], F32,
                                kind="ExternalOutput")

    with tile.TileContext(nc) as tc:
        with (
            tc.tile_pool(name="w", bufs=1) as wp,
            tc.tile_pool(name="tmp", bufs=1) as tp,
            tc.tile_pool(name="xs", bufs=4) as xsp,
            tc.tile_pool(name="xc0", bufs=1) as xc0p,
            tc.tile_pool(name="lat", bufs=1) as latp,
            tc.tile_pool(name="h", bufs=1) as hp,
            tc.tile_pool(name="elu", bufs=2) as ep,
            tc.tile_pool(name="cb", bufs=1) as cbp,
            tc.tile_pool(name="sm", bufs=1) as smp,
            tc.tile_pool(name="po", bufs=2) as pop,
            tc.tile_pool(name="ps", bufs=1, space="PSUM") as psp,
            tc.tile_pool(name="psg", bufs=2, space="PSUM") as psg,
            tc.tile_pool(name="pss", bufs=2, space="PSUM") as pss,
        ):
            # ---- persistent weight/activation tiles ----
            SM = wp.tile([128, 128 + 128 + E + 2 + 1 + 128 + OUT], F32)
            XX0f = tp.tile([128, (1 + L0C) * NS], F32, tag="big")
            W0 = wp.tile([128, NJ0 * H], BF16)
            W1 = wp.tile([128, NJ1 * H], BF16)
            W2 = wp.tile([128, NJ2 * OUT], BF16)

            # latency-critical small loads on the sync engine: its HWDGE
            # issues start ~4us before the gpsimd preamble finishes, and the
            # DMA engines are still empty so they land fast
            nc.sync.dma_start(SM[:], smalls_ext[:])
            nc.sync.dma_start(XX0f[:], xgx0_ext[:])
            # weight casts f32->bf16 on gpsimd SWDGE, zero-pad rows skipped
            W0v = W0[:].rearrange("p (j h) -> p j h", j=NJ0)
            W1v = W1[:].rearrange("p (j h) -> p j h", j=NJ1)
            # zero the untransferred pad rows (their rhs is zero, but
            # uninitialized bf16 could be NaN and poison 0*NaN in PSUM)
            nc.vector.memset(W0v[:, 1:NJ0:L0C, :], 0.0)
            nc.vector.memset(W0v[:, 2:NJ0:L0C, :], 0.0)
            nc.vector.memset(W1v[:, 4:NJ1:L1C, :], 0.0)
            nc.gpsimd.dma_start(
                W0v[:, 0:NJ0:L0C, :], w0a_ext[:])
            nc.gpsimd.dma_start(
                W0v[0:COND - 128, 1:NJ0:L0C, :], w0b_ext[:])
            nc.gpsimd.dma_start(
                W0v[0:LAT + E, 2:NJ0:L0C, :], w0c_ext[:])
            nc.gpsimd.dma_start(
                W1v[0:LAT + E, 4:NJ1:L1C, :], w1l_ext[:])
            W1h = W1[:].rearrange("p (e c h) -> p e c h", e=E, c=L1C)
            nc.gpsimd.dma_start(W1h[:, 0:4, 0:4, :], w1a_ext[:])
            nc.gpsimd.dma_start(W1h[:, 4:8, 0:4, :], w1b_ext[:])

            # slice views into the packed smalls tile
            GW1f = SM[0:GIN, 0:128]
            GW2f = SM[:, 128:256]
            GW3f = SM[:, 256:256 + E]
            GB12 = SM[:, 264:266]
            GB3 = SM[0:E, 266:267]
            IDT = SM[:, 267:395]
            B2f = SM[0:E, 395:395 + OUT]

            # bf16 converts (DVE for the small gate weights, ACT for inputs)
            GWB = wp.tile([128, 256 + E], BF16)
            GW1 = GWB[0:GIN, 0:128]
            GW2 = GWB[:, 128:256]
            GW3 = GWB[:, 256:256 + E]
            B2 = wp.tile([E, OUT], BF16)
            XX0 = wp.tile([128, (1 + L0C) * NS], BF16)
            XG = XX0[0:GIN, 0:NS]
            X0 = XX0[:, NS:(1 + L0C) * NS]
            nc.vector.tensor_copy(GWB[:], SM[:, 0:256 + E])
            nc.vector.tensor_copy(B2[:], B2f)
            nc.scalar.activation(XX0[:, 0:NS], XX0f[:, 0:NS], AF.Copy)
            nc.scalar.activation(XX0[:, NS:(1 + L0C) * NS],
                                 XX0f[:, NS:(1 + L0C) * NS], AF.Copy)

            # ---- gate network (feature-major, bf16 matmuls) ----
            def gate_elu(ps_in, out_tile, bias=0.0):
                e = ep.tile([128, NS], BF16, tag="elu_e")
                r = ep.tile([128, NS], BF16, tag="elu_r")
                nc.scalar.activation(e[:], ps_in, AF.Exp, bias=bias)
                nc.scalar.activation(r[:], ps_in, AF.Relu, bias=bias)
                nc.vector.scalar_tensor_tensor(out_tile, e[:], -1.0, r[:],
                                               ALU.add, ALU.min)

            pg1 = psg.tile([128, NS], F32, tag="pg")
            nc.tensor.matmul(pg1[:], GW1, XG, start=True, stop=True)
            G1 = tp.tile([128, NS], BF16, tag="g1")
            gate_elu(pg1[:], G1[:], GB12[:, 0:1])

            pg2 = psg.tile([128, NS], F32, tag="pg")
            nc.tensor.matmul(pg2[:], GW2, G1[:], start=True, stop=True)
            G2 = tp.tile([128, NS], BF16, tag="g2")
            gate_elu(pg2[:], G2[:], GB12[:, 1:2])

            pgl = pss.tile([E, NS], F32, tag="sm")
            nc.tensor.matmul(pgl[:], GW3, G2[:], start=True, stop=True)
            EXPF = smp.tile([E, NS], F32)
            nc.scalar.activation(EXPF[:], pgl[:], AF.Exp, bias=GB3)

            # softmax normalization, sample-major
            E4 = smp.tile([128, NS // 128, E], F32)
            for c in range(NS // 128):
                pt = pss.tile([128, E], F32, tag="sm")
                nc.tensor.transpose(pt[:], EXPF[:, c * 128:(c + 1) * 128],
                                    IDT[0:E, 0:E])
                nc.vector.tensor_copy(E4[:, c, :], pt[:])
            S4 = smp.tile([128, NS // 128], F32)
            nc.vector.tensor_reduce(S4[:], E4[:], mybir.AxisListType.X, ALU.add)
            R4 = smp.tile([128, NS // 128], F32)
            nc.vector.reciprocal(R4[:], S4[:])
            C4 = smp.tile([128, NS // 128, E], F32)
            for c in range(NS // 128):
                nc.vector.tensor_scalar_mul(C4[:, c, :], E4[:, c, :],
                                            R4[:, c:c + 1])
            nc.sync.dma_start(coeff4_ext[:], C4[:])

            # coeff^T [E, NS] bf16 (bias rows + L2 bias matmuls)
            CT = smp.tile([E, NS], BF16)
            for c in range(NS // 128):
                ptc = pss.tile([E, 128], F32, tag="sm")
                nc.tensor.transpose(ptc[:], C4[:, c, :], IDT[:, 0:128])
                nc.vector.tensor_copy(CT[:, c * 128:(c + 1) * 128], ptc[:])
            # flatten rows into partition 0, then DMA-broadcast each row to
            # all 128 partitions (no compute engine involved)
            CTS = smp.tile([1, E * NS], BF16)
            nc.sync.dma_start(CTS[:], CT[:])
            ONES = smp.tile([1, 128], BF16)
            nc.vector.memset(ONES[:], 1.0)
            CB = []
            for e in range(E):
                pcb = psg.tile([128, NS], F32, name=f"pcb{e}", tag="pg")
                nc.tensor.matmul(pcb[:], ONES[:],
                                 CTS[0:1, e * NS:(e + 1) * NS],
                                 start=True, stop=True)
                cb = cbp.tile([128, NS], BF16, tag=f"cb{e}")
                if e % 2 == 0:
                    nc.scalar.activation(cb[:], pcb[:], AF.Copy)
                else:
                    nc.vector.tensor_copy(cb[:], pcb[:])
                CB.append(cb)

            def cb4(e, k):
                return CB[e][:].unsqueeze(1).broadcast_to([128, k, NS])

            # W2 as raw f32 (shares the xx0f slot; converted on ACT later),
            # which shortens the gpsimd cast stream
            W2f = tp.tile([128, NJ2 * OUT], F32, tag="big")
            nc.sync.dma_start(W2f[:], w2_ext[:])
            nc.scalar.activation(W2[:], W2f[:], AF.Copy)

            # ---- layer 0 ----
            # chunk-major order matching weight-DMA arrival: all experts'
            # cond[0:128] chunks (w0a) first, then cond[128:] (w0b), then
            # latent+bias (w0c)
            XSLAT = []
            XC0 = []
            H1 = hp.tile([128, 4 * NS], BF16)
            ps_l0 = [psp.tile([128, NS], F32, name=f"psl0_{og}", tag=f"ps{og}")
                     for og in range(4)]
            for e in range(E):
                xc = xc0p.tile([128, 2 * NS], BF16, name=f"xc0_{e}", tag=f"xc0_{e}")
                nc.vector.tensor_mul(
                    xc[:].rearrange("p (c s) -> p c s", c=2),
                    X0[:, 0:2 * NS].rearrange("p (c s) -> p c s", c=2),
                    cb4(e, 2))
                XC0.append(xc)
            for e in range(E):
                t = latp.tile([128, NS], BF16, name=f"lat{e}", tag=f"lat{e}")
                nc.vector.tensor_mul(t[:], X0[:, 2 * NS:3 * NS], CB[e][:])
                XSLAT.append(t)
                if e == 0:
                    # bias rows: coeff^T into spare zero rows 64..71 of
                    # expert-0's latent chunk (w0h/w1h carry b0/b1 there)
                    nc.sync.dma_start(XSLAT[0][64:64 + E, :], CT[:])
            nfirst = True
            for c in range(L0C):
                for e in range(E):
                    j = e * L0C + c
                    rhs = (XC0[e][:, c * NS:(c + 1) * NS] if c < 2
                           else XSLAT[e][:])
                    for og in range(4):
                        nc.tensor.matmul(
                            ps_l0[og][:],
                            W0[:, j * H + og * 128: j * H + og * 128 + 128],
                            rhs, start=nfirst,
                            stop=(c == L0C - 1 and e == E - 1))
                    nfirst = False
            for og in range(4):
                gate_elu(ps_l0[og][:], H1[:, og * NS:(og + 1) * NS])

            # ---- layer 1 ----
            # latent chunks first: their inputs exist since layer 0, so the
            # PE crosses the L0->L1 boundary with zero stall while the L0
            # ELUs and the first L1 scale-TTs run on ACT/DVE.
            H2 = hp.tile([128, 4 * NS], BF16)
            ps_l1 = [psp.tile([128, NS], F32, name=f"psl1_{og}", tag=f"ps{og}")
                     for og in range(4)]
            for e in range(E):
                j = e * L1C + 4
                for og in range(4):
                    nc.tensor.matmul(
                        ps_l1[og][:],
                        W1[:, j * H + og * 128: j * H + og * 128 + 128],
                        XSLAT[e][:], start=(e == 0), stop=False)
            for e in range(E):
                xc = xsp.tile([128, 4 * NS], BF16, tag="xs")
                nc.vector.tensor_mul(
                    xc[:].rearrange("p (c s) -> p c s", c=4),
                    H1[:].rearrange("p (c s) -> p c s", c=4),
                    cb4(e, 4))
                if e < E - 1:
                    for c in range(4):
                        j = e * L1C + c
                        for og in range(4):
                            nc.tensor.matmul(
                                ps_l1[og][:],
                                W1[:, j * H + og * 128: j * H + og * 128 + 128],
                                xc[:, c * NS:(c + 1) * NS],
                                start=False, stop=False)
                else:
                    # last expert og-major so og0 finishes early and its ELU
                    # + L2 scale-TT overlap the remaining og matmuls
                    for og in range(4):
                        for c in range(4):
                            j = e * L1C + c
                            nc.tensor.matmul(
                                ps_l1[og][:],
                                W1[:, j * H + og * 128: j * H + og * 128 + 128],
                                xc[:, c * NS:(c + 1) * NS],
                                start=False, stop=(c == 3))
                        gate_elu(ps_l1[og][:], H2[:, og * NS:(og + 1) * NS])

            # ---- layer 2 (linear) ----
            ogs2 = ((0, 128), (128, OUT - 128))
            ps_l2 = [psp.tile([128, NS], F32, name=f"psl2_{og}", tag=f"ps{og}")
                     for og in range(2)]
            xcs2 = []
            for e in range(E):
                xc = xsp.tile([128, 4 * NS], BF16, tag="xs")
                nc.vector.tensor_mul(
                    xc[:].rearrange("p (c s) -> p c s", c=4),
                    H2[:].rearrange("p (c s) -> p c s", c=4),
                    cb4(e, 4))
                xcs2.append(xc)
                if e == E - 1:
                    break
                for c in range(L2C):
                    j = e * L2C + c
                    for og, (m0, msz) in enumerate(ogs2):
                        nc.tensor.matmul(
                            ps_l2[og][0:msz, :],
                            W2[:, j * OUT + m0: j * OUT + m0 + msz],
                            xc[:, c * NS:(c + 1) * NS],
                            start=(j == 0), stop=False)
            for og, (m0, msz) in enumerate(ogs2):
                e = E - 1
                for c in range(L2C):
                    j = e * L2C + c
                    nc.tensor.matmul(
                        ps_l2[og][0:msz, :],
                        W2[:, j * OUT + m0: j * OUT + m0 + msz],
                        xcs2[e][:, c * NS:(c + 1) * NS],
                        start=False, stop=False)
                nc.tensor.matmul(ps_l2[og][0:msz, :], B2[:, m0:m0 + msz], CT[:],
                                 start=False, stop=True)
                po = pop.tile([128, NS], F32, name=f"po{og}", tag="po")
                nc.scalar.activation(po[0:msz, :], ps_l2[og][0:msz, :], AF.Copy)
                nc.sync.dma_start(predt_ext[m0:m0 + msz, :], po[0:msz, :])

    nc.compile()
    return nc


def _prep(inputs):
    """Host-side shard/layout prep -> list of per-core input dicts."""
    lat = np.ascontiguousarray(np.asarray(inputs["latent"], dtype=np.float32))
    cond = np.ascontiguousarray(np.asarray(inputs["condition"], dtype=np.float32))
    ph = np.asarray(inputs["phase"], dtype=np.float32).reshape(N, PH2)
    gw1 = np.asarray(inputs["gate_w1"], dtype=np.float32)
    gw2 = np.asarray(inputs["gate_w2"], dtype=np.float32)
    gw3 = np.asarray(inputs["gate_w3"], dtype=np.float32)
    gb1 = np.asarray(inputs["gate_b1"], dtype=np.float32)
    gb2 = np.asarray(inputs["gate_b2"], dtype=np.float32)
    gb3 = np.asarray(inputs["gate_b3"], dtype=np.float32)
    w0 = np.asarray(inputs["w0"], dtype=np.float32)
    b0 = np.asarray(inputs["b0"], dtype=np.float32)
    w1 = np.asarray(inputs["w1"], dtype=np.float32)
    b1 = np.asarray(inputs["b1"], dtype=np.float32)
    w2 = np.asarray(inputs["w2"], dtype=np.float32)
    b2 = np.asarray(inputs["b2"], dtype=np.float32)

    xg_t = np.ascontiguousarray(np.concatenate([ph, lat], axis=1).T)  # [104, N]
    x0h = np.zeros((128, L0C, N), dtype=np.float32)
    x0h[:, 0, :] = cond.T[0:128]
    x0h[0:COND - 128, 1, :] = cond.T[128:COND]
    x0h[0:LAT, 2, :] = lat.T

    w0h = np.zeros((128, NJ0, H), dtype=np.float32)
    for e in range(E):
        w0h[:, e * L0C + 0] = w0[e, 0:128]
        w0h[0:COND - 128, e * L0C + 1] = w0[e, 128:COND]
        w0h[0:LAT, e * L0C + 2] = w0[e, COND:X0R]
    w0h[64:64 + E, 2] = b0  # bias rows ride expert-0's latent chunk
    w1h = np.zeros((128, NJ1, H), dtype=np.float32)
    for e in range(E):
        for c in range(4):
            w1h[:, e * L1C + c] = w1[e, c * 128:(c + 1) * 128]
        w1h[0:LAT, e * L1C + 4] = w1[e, H:H + LAT]
    w1h[64:64 + E, 4] = b1
    w2h = np.zeros((128, NJ2, OUT), dtype=np.float32)
    for e in range(E):
        for c in range(L2C):
            w2h[:, e * L2C + c] = w2[e, c * 128:(c + 1) * 128]

    # packed smalls: gw1|gw2|gw3|gb12|gb3|ident|b2
    smalls = np.zeros((128, 128 + 128 + E + 2 + 1 + 128 + OUT), dtype=np.float32)
    smalls[0:GIN, 0:128] = gw1
    smalls[:, 128:256] = gw2
    smalls[:, 256:256 + E] = gw3
    smalls[:, 264] = gb1
    smalls[:, 265] = gb2
    smalls[0:E, 266] = gb3
    smalls[:, 267:395] = np.eye(128, dtype=np.float32)
    smalls[0:E, 395:395 + OUT] = b2

    xgx0 = np.zeros((128, 1 + L0C, N), dtype=np.float32)
    xgx0[0:GIN, 0, :] = xg_t
    xgx0[:, 1:, :] = x0h

    w0v = w0h.reshape(128, E, L0C, H)
    w1v = w1h.reshape(128, E, L1C, H)
    shared = {
        "smalls": smalls,
        "w0a": np.ascontiguousarray(w0v[:, :, 0, :]),
        "w0b": np.ascontiguousarray(w0v[0:COND - 128, :, 1, :]),
        "w0c": np.ascontiguousarray(w0v[0:LAT + E, :, 2, :]),
        "w1l": np.ascontiguousarray(w1v[0:LAT + E, :, 4, :]),
        "w1a": np.ascontiguousarray(w1v[:, 0:4, 0:4, :]),
        "w1b": np.ascontiguousarray(w1v[:, 4:8, 0:4, :]),
        "w2": w2h,
    }
    in_maps = []
    for ci in range(NCORES):
        s = slice(ci * NS, (ci + 1) * NS)
        m = {"xgx0": np.ascontiguousarray(xgx0[:, :, s])}
        m.update(shared)
        in_maps.append(m)
    return in_maps


def kernel(**inputs):
    if "nc" not in _CACHE:
        _CACHE["nc"] = build()
    nc = _CACHE["nc"]
    in_maps = _prep(inputs)
    res = run_bass_kernel_spmd(nc, in_maps, core_ids=list(range(NCORES)))
    preds, coeffs = [], []
    for ci in range(NCORES):
        r = res.results[ci]
        preds.append(np.ascontiguousarray(r["predt"].T))          # [NS, OUT]
        c4 = r["coeff4"]                                          # [128, 4, E]
        coeffs.append(np.ascontiguousarray(
            c4.transpose(1, 0, 2).reshape(NS, E)))
    pred = np.concatenate(preds, axis=0).astype(np.float32)
    coeff = np.concatenate(coeffs, axis=0).astype(np.float32)
    return pred, coeff


# revision 23
# speedup vs baseline: 1.1618x; 1.1618x over previous
"""Trainium2 Bass kernel for nn_Application_85469849191065 (moe_routing).

Data-parallel over the batch dim N=4096 across 8 NeuronCores (512 samples
per core); the small expert weight banks are replicated to every core.

On-device layout is feature-major (activations stored transposed,
[features, samples]) so every matmul's contraction dim lands on the
partition axis with no per-layer transposes.  The per-sample expert blend
  out_n = sum_e coeff[n,e] * (x_n @ W[e] + b[e])
is computed as scale-input MoE: the moving operand of each contraction
chunk is (coeff_e broadcast across partitions) * X^T, so the blend over
experts accumulates for free in PSUM.  The bias term rides spare zero
rows of the latent contraction chunk whose moving rows hold coeff^T.
Compute dtype is bf16 (weights cast f32->bf16 in-flight by the gpsimd
DMA engines), accumulation fp32 in PSUM.
"""

import numpy as np

import concourse.bass as bass
import concourse.bacc as bacc
import concourse.mybir as mybir
import concourse.tile as tile
from concourse.bass_utils import run_bass_kernel_spmd

F32 = mybir.dt.float32
BF16 = mybir.dt.bfloat16
AF = mybir.ActivationFunctionType
ALU = mybir.AluOpType

N = 4096
NCORES = 8
NS = N // NCORES  # 512 samples per core
LAT, COND, PH2, E = 64, 219, 40, 8
H, OUT = 512, 171
GIN = PH2 + LAT  # 104
X0R = COND + LAT  # 283

L0C = 3   # per-expert L0 chunks: cond[0:128], cond[128:219], latent(64)
L1C = 5   # 4 h chunks + the shared latent chunk
L2C = 4
NJ0 = E * L0C  # 24
NJ1 = E * L1C  # 40
NJ2 = E * L2C  # 32

_CACHE = {}


def build():
    nc = bacc.Bacc("TRN2", target_bir_lowering=False, debug=False,
                   num_devices=NCORES)

    # ---- dram parameters (per-core shard shapes) ----
    # xgx0: chunk0 = gate input (104 rows), chunks 1..3 = x0 chunks
    xgx0_ext = nc.dram_tensor("xgx0", [128, 1 + L0C, NS], F32,
                              kind="ExternalInput")
    # smalls: gw1|gw2|gw3|gb12|gb3|ident|b2  packed along free dim
    smalls_ext = nc.dram_tensor("smalls", [128, 128 + 128 + E + 2 + 1 + 128 + OUT],
                                F32, kind="ExternalInput")
    # trimmed weight groups (no zero-pad rows on the wire)
    w0a_ext = nc.dram_tensor("w0a", [128, E, H], F32, kind="ExternalInput")
    w0b_ext = nc.dram_tensor("w0b", [128, E, H], F32, kind="ExternalInput")
    w0c_ext = nc.dram_tensor("w0c", [128, E, H], F32, kind="ExternalInput")
    w1l_ext = nc.dram_tensor("w1l", [128, E, H], F32, kind="ExternalInput")
    w1a_ext = nc.dram_tensor("w1a", [128, 4, 4, H], F32, kind="ExternalInput")
    w1b_ext = nc.dram_tensor("w1b", [128, 4, 4, H], F32, kind="ExternalInput")
    w2_ext = nc.dram_tensor("w2", [128, NJ2, OUT], F32, kind="ExternalInput")
    predt_ext = nc.dram_tensor("predt", [OUT, NS], F32, kind="ExternalOutput")
    coeff4_ext = nc.dram_tensor("coeff4", [128, NS // 128, E], F32,
                                kind="ExternalOutput")

    with tile.TileContext(nc) as tc:
        with (
            tc.tile_pool(name="w", bufs=1) as wp,
            tc.tile_pool(name="tmp", bufs=1) as tp,
            tc.tile_pool(name="xs", bufs=4) as xsp,
            tc.tile_pool(name="xc0", bufs=1) as xc0p,
            tc.tile_pool(name="lat", bufs=1) as latp,
            tc.tile_pool(name="h", bufs=1) as hp,
            tc.tile_pool(name="elu", bufs=2) as ep,
            tc.tile_pool(name="cb", bufs=1) as cbp,
            tc.tile_pool(name="sm", bufs=1) as smp,
            tc.tile_pool(name="po", bufs=2) as pop,
            tc.tile_pool(name="ps", bufs=1, space="PSUM") as psp,
            tc.tile_pool(name="psg", bufs=2, space="PSUM") as psg,
            tc.tile_pool(name="pss", bufs=2, space="PSUM") as pss,
        ):
            # ---- persistent weight/activation tiles ----
            SM = wp.tile([128, 128 + 128 + E + 2 + 1 + 128 + OUT], F32)
            XX0f = tp.tile([128, (1 + L0C) * NS], F32, tag="big")
            W0 = wp.tile([128, NJ0 * H], BF16)
            W1 = wp.tile([128, NJ1 * H], BF16)
            W2 = wp.tile([128, NJ2 * OUT], BF16)

            # latency-critical small loads on the sync engine: its HWDGE
            # issues start ~4us before the gpsimd preamble finishes, and the
            # DMA engines are still empty so they land fast
            nc.sync.dma_start(SM[:], smalls_ext[:])
            nc.sync.dma_start(XX0f[:], xgx0_ext[:])
            # weight casts f32->bf16 on gpsimd SWDGE, zero-pad rows skipped
            W0v = W0[:].rearrange("p (j h) -> p j h", j=NJ0)
            W1v = W1[:].rearrange("p (j h) -> p j h", j=NJ1)
            nc.gpsimd.dma_start(W0v[:, 0:NJ0:L0C, :], w0a_ext[:])
            nc.gpsimd.dma_start(W0v[:, 1:NJ0:L0C, :], w0b_ext[:])
            nc.gpsimd.dma_start(W0v[:, 2:NJ0:L0C, :], w0c_ext[:])
            nc.gpsimd.dma_start(W1v[:, 4:NJ1:L1C, :], w1l_ext[:])
            W1h = W1[:].rearrange("p (e c h) -> p e c h", e=E, c=L1C)
            nc.gpsimd.dma_start(W1h[:, 0:4, 0:4, :], w1a_ext[:])
            nc.gpsimd.dma_start(W1h[:, 4:8, 0:4, :], w1b_ext[:])

            # slice views into the packed smalls tile
            GW1f = SM[0:GIN, 0:128]
            GW2f = SM[:, 128:256]
            GW3f = SM[:, 256:256 + E]
            GB12 = SM[:, 264:266]
            GB3 = SM[0:E, 266:267]
            IDT = SM[:, 267:395]
            B2f = SM[0:E, 395:395 + OUT]

            # bf16 converts (DVE for the small gate weights, ACT for inputs)
            GWB = wp.tile([128, 256 + E], BF16)
            GW1 = GWB[0:GIN, 0:128]
            GW2 = GWB[:, 128:256]
            GW3 = GWB[:, 256:256 + E]
            B2 = wp.tile([E, OUT], BF16)
            XX0 = wp.tile([128, (1 + L0C) * NS], BF16)
            XG = XX0[0:GIN, 0:NS]
            X0 = XX0[:, NS:(1 + L0C) * NS]
            nc.vector.tensor_copy(GWB[:], SM[:, 0:256 + E])
            nc.vector.tensor_copy(B2[:], B2f)
            nc.scalar.activation(XX0[:, 0:NS], XX0f[:, 0:NS], AF.Copy)
            nc.scalar.activation(XX0[:, NS:(1 + L0C) * NS],
                                 XX0f[:, NS:(1 + L0C) * NS], AF.Copy)

            # ---- gate network (feature-major, bf16 matmuls) ----
            def gate_elu(ps_in, out_tile, bias=0.0):
                e = ep.tile([128, NS], BF16, tag="elu_e")
                r = ep.tile([128, NS], BF16, tag="elu_r")
                nc.scalar.activation(e[:], ps_in, AF.Exp, bias=bias)
                nc.scalar.activation(r[:], ps_in, AF.Relu, bias=bias)
                nc.vector.scalar_tensor_tensor(out_tile, e[:], -1.0, r[:],
                                               ALU.add, ALU.min)

            pg1 = psg.tile([128, NS], F32, tag="pg")
            nc.tensor.matmul(pg1[:], GW1, XG, start=True, stop=True)
            G1 = tp.tile([128, NS], BF16, tag="g1")
            gate_elu(pg1[:], G1[:], GB12[:, 0:1])

            pg2 = psg.tile([128, NS], F32, tag="pg")
            nc.tensor.matmul(pg2[:], GW2, G1[:], start=True, stop=True)
            G2 = tp.tile([128, NS], BF16, tag="g2")
            gate_elu(pg2[:], G2[:], GB12[:, 1:2])

            pgl = pss.tile([E, NS], F32, tag="sm")
            nc.tensor.matmul(pgl[:], GW3, G2[:], start=True, stop=True)
            EXPF = smp.tile([E, NS], F32)
            nc.scalar.activation(EXPF[:], pgl[:], AF.Exp, bias=GB3)

            # softmax normalization, sample-major
            E4 = smp.tile([128, NS // 128, E], F32)
            for c in range(NS // 128):
                pt = pss.tile([128, E], F32, tag="sm")
                nc.tensor.transpose(pt[:], EXPF[:, c * 128:(c + 1) * 128],
                                    IDT[0:E, 0:E])
                nc.vector.tensor_copy(E4[:, c, :], pt[:])
            S4 = smp.tile([128, NS // 128], F32)
            nc.vector.tensor_reduce(S4[:], E4[:], mybir.AxisListType.X, ALU.add)
            R4 = smp.tile([128, NS // 128], F32)
            nc.vector.reciprocal(R4[:], S4[:])
            C4 = smp.tile([128, NS // 128, E], F32)
            for c in range(NS // 128):
                nc.vector.tensor_scalar_mul(C4[:, c, :], E4[:, c, :],
                                            R4[:, c:c + 1])
            nc.sync.dma_start(coeff4_ext[:], C4[:])

            # coeff^T [E, NS] bf16 (bias rows + L2 bias matmuls)
            CT = smp.tile([E, NS], BF16)
            for c in range(NS // 128):
                ptc = pss.tile([E, 128], F32, tag="sm")
                nc.tensor.transpose(ptc[:], C4[:, c, :], IDT[:, 0:128])
                nc.vector.tensor_copy(CT[:, c * 128:(c + 1) * 128], ptc[:])
            # flatten rows into partition 0, then DMA-broadcast each row to
            # all 128 partitions (no compute engine involved)
            CTS = smp.tile([1, E * NS], BF16)
            nc.sync.dma_start(CTS[:], CT[:])
            ONES = smp.tile([1, 128], BF16)
            nc.vector.memset(ONES[:], 1.0)
            CB = []
            for e in range(E):
                pcb = psg.tile([128, NS], F32, name=f"pcb{e}", tag="pg")
                nc.tensor.matmul(pcb[:], ONES[:],
                                 CTS[0:1, e * NS:(e + 1) * NS],
                                 start=True, stop=True)
                cb = cbp.tile([128, NS], BF16, tag=f"cb{e}")
                if e % 2 == 0:
                    nc.scalar.activation(cb[:], pcb[:], AF.Copy)
                else:
                    nc.vector.tensor_copy(cb[:], pcb[:])
                CB.append(cb)

            def cb4(e, k):
                return CB[e][:].unsqueeze(1).broadcast_to([128, k, NS])

            # W2 as raw f32 (shares the xx0f slot; converted on ACT later),
            # which shortens the gpsimd cast stream
            W2f = tp.tile([128, NJ2 * OUT], F32, tag="big")
            nc.sync.dma_start(W2f[:], w2_ext[:])
            nc.scalar.activation(W2[:], W2f[:], AF.Copy)

            # ---- layer 0 ----
            # chunk-major order matching weight-DMA arrival: all experts'
            # cond[0:128] chunks (w0a) first, then cond[128:] (w0b), then
            # latent+bias (w0c)
            XSLAT = []
            XC0 = []
            H1 = hp.tile([128, 4 * NS], BF16)
            ps_l0 = [psp.tile([128, NS], F32, name=f"psl0_{og}", tag=f"ps{og}")
                     for og in range(4)]
            for e in range(E):
                xc = xc0p.tile([128, 2 * NS], BF16, name=f"xc0_{e}", tag=f"xc0_{e}")
                nc.vector.tensor_mul(
                    xc[:].rearrange("p (c s) -> p c s", c=2),
                    X0[:, 0:2 * NS].rearrange("p (c s) -> p c s", c=2),
                    cb4(e, 2))
                XC0.append(xc)
            for e in range(E):
                t = latp.tile([128, NS], BF16, name=f"lat{e}", tag=f"lat{e}")
                nc.vector.tensor_mul(t[:], X0[:, 2 * NS:3 * NS], CB[e][:])
                XSLAT.append(t)
                if e == 0:
                    # bias rows: coeff^T into spare zero rows 64..71 of
                    # expert-0's latent chunk (w0h/w1h carry b0/b1 there)
                    nc.sync.dma_start(XSLAT[0][64:64 + E, :], CT[:])
            nfirst = True
            for c in range(L0C):
                for e in range(E):
                    j = e * L0C + c
                    rhs = (XC0[e][:, c * NS:(c + 1) * NS] if c < 2
                           else XSLAT[e][:])
                    for og in range(4):
                        nc.tensor.matmul(
                            ps_l0[og][:],
                            W0[:, j * H + og * 128: j * H + og * 128 + 128],
                            rhs, start=nfirst,
                            stop=(c == L0C - 1 and e == E - 1))
                    nfirst = False
            for og in range(4):
                gate_elu(ps_l0[og][:], H1[:, og * NS:(og + 1) * NS])

            # ---- layer 1 ----
            # latent chunks first: their inputs exist since layer 0, so the
            # PE crosses the L0->L1 boundary with zero stall while the L0
            # ELUs and the first L1 scale-TTs run on ACT/DVE.
            H2 = hp.tile([128, 4 * NS], BF16)
            ps_l1 = [psp.tile([128, NS], F32, name=f"psl1_{og}", tag=f"ps{og}")
                     for og in range(4)]
            for e in range(E):
                j = e * L1C + 4
                for og in range(4):
                    nc.tensor.matmul(
                        ps_l1[og][:],
                        W1[:, j * H + og * 128: j * H + og * 128 + 128],
                        XSLAT[e][:], start=(e == 0), stop=False)
            for e in range(E):
                xc = xsp.tile([128, 4 * NS], BF16, tag="xs")
                nc.vector.tensor_mul(
                    xc[:].rearrange("p (c s) -> p c s", c=4),
                    H1[:].rearrange("p (c s) -> p c s", c=4),
                    cb4(e, 4))
                if e < E - 1:
                    for c in range(4):
                        j = e * L1C + c
                        for og in range(4):
                            nc.tensor.matmul(
                                ps_l1[og][:],
                                W1[:, j * H + og * 128: j * H + og * 128 + 128],
                                xc[:, c * NS:(c + 1) * NS],
                                start=False, stop=False)
                else:
                    # last expert og-major so og0 finishes early and its ELU
                    # + L2 scale-TT overlap the remaining og matmuls
                    for og in range(4):
                        for c in range(4):
                            j = e * L1C + c
                            nc.tensor.matmul(
                                ps_l1[og][:],
                                W1[:, j * H + og * 128: j * H + og * 128 + 128],
                                xc[:, c * NS:(c + 1) * NS],
                                start=False, stop=(c == 3))
                        gate_elu(ps_l1[og][:], H2[:, og * NS:(og + 1) * NS])

            # ---- layer 2 (linear) ----
            ogs2 = ((0, 128), (128, OUT - 128))
            ps_l2 = [psp.tile([128, NS], F32, name=f"psl2_{og}", tag=f"ps{og}")
                     for og in range(2)]
            xcs2 = []
            for e in range(E):
                xc = xsp.tile([128, 4 * NS], BF16, tag="xs")
                nc.vector.tensor_mul(
                    xc[:].rearrange("p (c s) -> p c s", c=4),
                    H2[:].rearrange("p (c s) -> p c s", c=4),
                    cb4(e, 4))
                xcs2.append(xc)
                if e == E - 1:
                    break
                for c in range(L2C):
                    j = e * L2C + c
                    for og, (m0, msz) in enumerate(ogs2):
                        nc.tensor.matmul(
                            ps_l2[og][0:msz, :],
                            W2[:, j * OUT + m0: j * OUT + m0 + msz],
                            xc[:, c * NS:(c + 1) * NS],
                            start=(j == 0), stop=False)
            for og, (m0, msz) in enumerate(ogs2):
                e = E - 1
                for c in range(L2C):
                    j = e * L2C + c
                    nc.tensor.matmul(
                        ps_l2[og][0:msz, :],
                        W2[:, j * OUT + m0: j * OUT + m0 + msz],
                        xcs2[e][:, c * NS:(c + 1) * NS],
                        start=False, stop=False)
                nc.tensor.matmul(ps_l2[og][0:msz, :], B2[:, m0:m0 + msz], CT[:],
                                 start=False, stop=True)
                po = pop.tile([128, NS], F32, name=f"po{og}", tag="po")
                nc.scalar.activation(po[0:msz, :], ps_l2[og][0:msz, :], AF.Copy)
                nc.sync.dma_start(predt_ext[m0:m0 + msz, :], po[0:msz, :])

    nc.compile()
    return nc


def _prep(inputs):
    """Host-side shard/layout prep -> list of per-core input dicts."""
    lat = np.ascontiguousarray(np.asarray(inputs["latent"], dtype=np.float32))
    cond = np.ascontiguousarray(np.asarray(inputs["condition"], dtype=np.float32))
    ph = np.asarray(inputs["phase"], dtype=np.float32).reshape(N, PH2)
    gw1 = np.asarray(inputs["gate_w1"], dtype=np.float32)
    gw2 = np.asarray(inputs["gate_w2"], dtype=np.float32)
    gw3 = np.asarray(inputs["gate_w3"], dtype=np.float32)
    gb1 = np.asarray(inputs["gate_b1"], dtype=np.float32)
    gb2 = np.asarray(inputs["gate_b2"], dtype=np.float32)
    gb3 = np.asarray(inputs["gate_b3"], dtype=np.float32)
    w0 = np.asarray(inputs["w0"], dtype=np.float32)
    b0 = np.asarray(inputs["b0"], dtype=np.float32)
    w1 = np.asarray(inputs["w1"], dtype=np.float32)
    b1 = np.asarray(inputs["b1"], dtype=np.float32)
    w2 = np.asarray(inputs["w2"], dtype=np.float32)
    b2 = np.asarray(inputs["b2"], dtype=np.float32)

    xg_t = np.ascontiguousarray(np.concatenate([ph, lat], axis=1).T)  # [104, N]
    x0h = np.zeros((128, L0C, N), dtype=np.float32)
    x0h[:, 0, :] = cond.T[0:128]
    x0h[0:COND - 128, 1, :] = cond.T[128:COND]
    x0h[0:LAT, 2, :] = lat.T

    w0h = np.zeros((128, NJ0, H), dtype=np.float32)
    for e in range(E):
        w0h[:, e * L0C + 0] = w0[e, 0:128]
        w0h[0:COND - 128, e * L0C + 1] = w0[e, 128:COND]
        w0h[0:LAT, e * L0C + 2] = w0[e, COND:X0R]
    w0h[64:64 + E, 2] = b0  # bias rows ride expert-0's latent chunk
    w1h = np.zeros((128, NJ1, H), dtype=np.float32)
    for e in range(E):
        for c in range(4):
            w1h[:, e * L1C + c] = w1[e, c * 128:(c + 1) * 128]
        w1h[0:LAT, e * L1C + 4] = w1[e, H:H + LAT]
    w1h[64:64 + E, 4] = b1
    w2h = np.zeros((128, NJ2, OUT), dtype=np.float32)
    for e in range(E):
        for c in range(L2C):
            w2h[:, e * L2C + c] = w2[e, c * 128:(c + 1) * 128]

    # packed smalls: gw1|gw2|gw3|gb12|gb3|ident|b2
    smalls = np.zeros((128, 128 + 128 + E + 2 + 1 + 128 + OUT), dtype=np.float32)
    smalls[0:GIN, 0:128] = gw1
    smalls[:, 128:256] = gw2
    smalls[:, 256:256 + E] = gw3
    smalls[:, 264] = gb1
    smalls[:, 265] = gb2
    smalls[0:E, 266] = gb3
    smalls[:, 267:395] = np.eye(128, dtype=np.float32)
    smalls[0:E, 395:395 + OUT] = b2

    xgx0 = np.zeros((128, 1 + L0C, N), dtype=np.float32)
    xgx0[0:GIN, 0, :] = xg_t
    xgx0[:, 1:, :] = x0h

    w0v = w0h.reshape(128, E, L0C, H)
    w1v = w1h.reshape(128, E, L1C, H)
    shared = {
        "smalls": smalls,
        "w0a": np.ascontiguousarray(w0v[:, :, 0, :]),
        "w0b": np.ascontiguousarray(w0v[:, :, 1, :]),
        "w0c": np.ascontiguousarray(w0v[:, :, 2, :]),
        "w1l": np.ascontiguousarray(w1v[:, :, 4, :]),
        "w1a": np.ascontiguousarray(w1v[:, 0:4, 0:4, :]),
        "w1b": np.ascontiguousarray(w1v[:, 4:8, 0:4, :]),
        "w2": w2h,
    }
    in_maps = []
    for ci in range(NCORES):
        s = slice(ci * NS, (ci + 1) * NS)
        m = {"xgx0": np.ascontiguousarray(xgx0[:, :, s])}
        m.update(shared)
        in_maps.append(m)
    return in_maps


def kernel(**inputs):
    if "nc" not in _CACHE:
        _CACHE["nc"] = build()
    nc = _CACHE["nc"]
    in_maps = _prep(inputs)
    res = run_bass_kernel_spmd(nc, in_maps, core_ids=list(range(NCORES)))
    preds, coeffs = [], []
    for ci in range(NCORES):
        r = res.results[ci]
        preds.append(np.ascontiguousarray(r["predt"].T))          # [NS, OUT]
        c4 = r["coeff4"]                                          # [128, 4, E]
        coeffs.append(np.ascontiguousarray(
            c4.transpose(1, 0, 2).reshape(NS, E)))
    pred = np.concatenate(preds, axis=0).astype(np.float32)
    coeff = np.concatenate(coeffs, axis=0).astype(np.float32)
    return pred, coeff


# revision 24
# speedup vs baseline: 1.2285x; 1.0575x over previous
"""Trainium2 Bass kernel for nn_Application_85469849191065 (moe_routing).

Data-parallel over the batch dim N=4096 across 8 NeuronCores (512 samples
per core); the small expert weight banks are replicated to every core.

On-device layout is feature-major (activations stored transposed,
[features, samples]) so every matmul's contraction dim lands on the
partition axis with no per-layer transposes.  The per-sample expert blend
  out_n = sum_e coeff[n,e] * (x_n @ W[e] + b[e])
is computed as scale-input MoE: the moving operand of each contraction
chunk is (coeff_e broadcast across partitions) * X^T, so the blend over
experts accumulates for free in PSUM.  The bias term rides spare zero
rows of the latent contraction chunk whose moving rows hold coeff^T.
Compute dtype is bf16 (weights cast f32->bf16 in-flight by the gpsimd
DMA engines), accumulation fp32 in PSUM.
"""

import numpy as np

import concourse.bass as bass
import concourse.bacc as bacc
import concourse.mybir as mybir
import concourse.tile as tile
from concourse.bass_utils import run_bass_kernel_spmd

F32 = mybir.dt.float32
BF16 = mybir.dt.bfloat16
AF = mybir.ActivationFunctionType
ALU = mybir.AluOpType

N = 4096
NCORES = 8
NS = N // NCORES  # 512 samples per core
LAT, COND, PH2, E = 64, 219, 40, 8
H, OUT = 512, 171
GIN = PH2 + LAT  # 104
X0R = COND + LAT  # 283

L0C = 3   # per-expert L0 chunks: cond[0:128], cond[128:219], latent(64)
L1C = 5   # 4 h chunks + the shared latent chunk
L2C = 4
NJ0 = E * L0C  # 24
NJ1 = E * L1C  # 40
NJ2 = E * L2C  # 32

_CACHE = {}


def build():
    nc = bacc.Bacc("TRN2", target_bir_lowering=False, debug=False,
                   num_devices=NCORES)

    # ---- dram parameters (per-core shard shapes) ----
    # xgx0: chunk0 = gate input (104 rows), chunks 1..3 = x0 chunks
    xgx0_ext = nc.dram_tensor("xgx0", [128, 1 + L0C, NS], F32,
                              kind="ExternalInput")
    # smalls: gw1|gw2|gw3|gb12|gb3|ident|b2  packed along free dim
    smalls_ext = nc.dram_tensor("smalls", [128, 128 + 128 + E + 2 + 1 + 128 + OUT],
                                F32, kind="ExternalInput")
    # trimmed weight groups (no zero-pad rows on the wire)
    w0a_ext = nc.dram_tensor("w0a", [128, E, H], F32, kind="ExternalInput")
    w0b_ext = nc.dram_tensor("w0b", [128, E, H], F32, kind="ExternalInput")
    w0c_ext = nc.dram_tensor("w0c", [128, E, H], F32, kind="ExternalInput")
    w1l_ext = nc.dram_tensor("w1l", [128, E, H], F32, kind="ExternalInput")
    w1a_ext = nc.dram_tensor("w1a", [128, 4, 4, H], F32, kind="ExternalInput")
    w1b_ext = nc.dram_tensor("w1b", [128, 4, 4, H], F32, kind="ExternalInput")
    w2_ext = nc.dram_tensor("w2", [128, NJ2, OUT], F32, kind="ExternalInput")
    predt_ext = nc.dram_tensor("predt", [OUT, NS], F32, kind="ExternalOutput")
    coeff4_ext = nc.dram_tensor("coeff4", [128, NS // 128, E], F32,
                                kind="ExternalOutput")

    with tile.TileContext(nc) as tc:
        with (
            tc.tile_pool(name="w", bufs=1) as wp,
            tc.tile_pool(name="tmp", bufs=1) as tp,
            tc.tile_pool(name="xs", bufs=4) as xsp,
            tc.tile_pool(name="xc0", bufs=1) as xc0p,
            tc.tile_pool(name="lat", bufs=1) as latp,
            tc.tile_pool(name="h", bufs=1) as hp,
            tc.tile_pool(name="elu", bufs=2) as ep,
            tc.tile_pool(name="cb", bufs=1) as cbp,
            tc.tile_pool(name="sm", bufs=1) as smp,
            tc.tile_pool(name="po", bufs=2) as pop,
            tc.tile_pool(name="ps", bufs=1, space="PSUM") as psp,
            tc.tile_pool(name="psg", bufs=2, space="PSUM") as psg,
            tc.tile_pool(name="pss", bufs=2, space="PSUM") as pss,
        ):
            # ---- persistent weight/activation tiles ----
            SM = wp.tile([128, 128 + 128 + E + 2 + 1 + 128 + OUT], F32)
            XX0f = tp.tile([128, (1 + L0C) * NS], F32, tag="big")
            W0 = wp.tile([128, NJ0 * H], BF16)
            W1 = wp.tile([128, NJ1 * H], BF16)
            W2 = wp.tile([128, NJ2 * OUT], BF16)

            # latency-critical small loads on the sync engine: its HWDGE
            # issues start ~4us before the gpsimd preamble finishes, and the
            # DMA engines are still empty so they land fast
            nc.sync.dma_start(SM[:], smalls_ext[:])
            nc.sync.dma_start(XX0f[:], xgx0_ext[:])
            # weight casts f32->bf16 on gpsimd SWDGE, zero-pad rows skipped
            W0v = W0[:].rearrange("p (j h) -> p j h", j=NJ0)
            W1v = W1[:].rearrange("p (j h) -> p j h", j=NJ1)
            half = 4 * L0C
            nc.gpsimd.dma_start(W0v[:, 0:half:L0C, :], w0a_ext[:, 0:4, :])
            nc.gpsimd.dma_start(W0v[:, half:NJ0:L0C, :], w0a_ext[:, 4:8, :])
            nc.gpsimd.dma_start(W0v[:, 1:half:L0C, :], w0b_ext[:, 0:4, :])
            nc.gpsimd.dma_start(W0v[:, half + 1:NJ0:L0C, :], w0b_ext[:, 4:8, :])
            nc.gpsimd.dma_start(W0v[:, 2:half:L0C, :], w0c_ext[:, 0:4, :])
            nc.gpsimd.dma_start(W0v[:, half + 2:NJ0:L0C, :], w0c_ext[:, 4:8, :])
            nc.gpsimd.dma_start(W1v[:, 4:NJ1:L1C, :], w1l_ext[:])
            W1h = W1[:].rearrange("p (e c h) -> p e c h", e=E, c=L1C)
            nc.gpsimd.dma_start(W1h[:, 0:4, 0:4, :], w1a_ext[:])
            nc.gpsimd.dma_start(W1h[:, 4:8, 0:4, :], w1b_ext[:])

            # slice views into the packed smalls tile
            GW1f = SM[0:GIN, 0:128]
            GW2f = SM[:, 128:256]
            GW3f = SM[:, 256:256 + E]
            GB12 = SM[:, 264:266]
            GB3 = SM[0:E, 266:267]
            IDT = SM[:, 267:395]
            B2f = SM[0:E, 395:395 + OUT]

            # bf16 converts (DVE for the small gate weights, ACT for inputs)
            GWB = wp.tile([128, 256 + E], BF16)
            GW1 = GWB[0:GIN, 0:128]
            GW2 = GWB[:, 128:256]
            GW3 = GWB[:, 256:256 + E]
            B2 = wp.tile([E, OUT], BF16)
            XX0 = wp.tile([128, (1 + L0C) * NS], BF16)
            XG = XX0[0:GIN, 0:NS]
            X0 = XX0[:, NS:(1 + L0C) * NS]
            nc.vector.tensor_copy(GWB[:], SM[:, 0:256 + E])
            nc.vector.tensor_copy(B2[:], B2f)
            nc.scalar.activation(XX0[:, 0:NS], XX0f[:, 0:NS], AF.Copy)
            nc.scalar.activation(XX0[:, NS:(1 + L0C) * NS],
                                 XX0f[:, NS:(1 + L0C) * NS], AF.Copy)

            # ---- gate network (feature-major, bf16 matmuls) ----
            def gate_elu(ps_in, out_tile, bias=0.0):
                e = ep.tile([128, NS], BF16, tag="elu_e")
                r = ep.tile([128, NS], BF16, tag="elu_r")
                nc.scalar.activation(e[:], ps_in, AF.Exp, bias=bias)
                nc.scalar.activation(r[:], ps_in, AF.Relu, bias=bias)
                nc.vector.scalar_tensor_tensor(out_tile, e[:], -1.0, r[:],
                                               ALU.add, ALU.min)

            pg1 = psg.tile([128, NS], F32, tag="pg")
            nc.tensor.matmul(pg1[:], GW1, XG, start=True, stop=True)
            G1 = tp.tile([128, NS], BF16, tag="g1")
            gate_elu(pg1[:], G1[:], GB12[:, 0:1])

            pg2 = psg.tile([128, NS], F32, tag="pg")
            nc.tensor.matmul(pg2[:], GW2, G1[:], start=True, stop=True)
            G2 = tp.tile([128, NS], BF16, tag="g2")
            gate_elu(pg2[:], G2[:], GB12[:, 1:2])

            pgl = pss.tile([E, NS], F32, tag="sm")
            nc.tensor.matmul(pgl[:], GW3, G2[:], start=True, stop=True)
            EXPF = smp.tile([E, NS], F32)
            nc.scalar.activation(EXPF[:], pgl[:], AF.Exp, bias=GB3)

            # softmax normalization, sample-major
            E4 = smp.tile([128, NS // 128, E], F32)
            for c in range(NS // 128):
                pt = pss.tile([128, E], F32, tag="sm")
                nc.tensor.transpose(pt[:], EXPF[:, c * 128:(c + 1) * 128],
                                    IDT[0:E, 0:E])
                nc.vector.tensor_copy(E4[:, c, :], pt[:])
            S4 = smp.tile([128, NS // 128], F32)
            nc.vector.tensor_reduce(S4[:], E4[:], mybir.AxisListType.X, ALU.add)
            R4 = smp.tile([128, NS // 128], F32)
            nc.vector.reciprocal(R4[:], S4[:])
            C4 = smp.tile([128, NS // 128, E], F32)
            for c in range(NS // 128):
                nc.vector.tensor_scalar_mul(C4[:, c, :], E4[:, c, :],
                                            R4[:, c:c + 1])
            nc.sync.dma_start(coeff4_ext[:], C4[:])

            # coeff^T [E, NS] bf16 (bias rows + L2 bias matmuls)
            CT = smp.tile([E, NS], BF16)
            for c in range(NS // 128):
                ptc = pss.tile([E, 128], F32, tag="sm")
                nc.tensor.transpose(ptc[:], C4[:, c, :], IDT[:, 0:128])
                nc.vector.tensor_copy(CT[:, c * 128:(c + 1) * 128], ptc[:])
            # flatten rows into partition 0, then DMA-broadcast each row to
            # all 128 partitions (no compute engine involved)
            CTS = smp.tile([1, E * NS], BF16)
            nc.sync.dma_start(CTS[:], CT[:])
            ONES = smp.tile([1, 128], BF16)
            nc.vector.memset(ONES[:], 1.0)
            CB = []
            for e in range(E):
                pcb = psg.tile([128, NS], F32, name=f"pcb{e}", tag="pg")
                nc.tensor.matmul(pcb[:], ONES[:],
                                 CTS[0:1, e * NS:(e + 1) * NS],
                                 start=True, stop=True)
                cb = cbp.tile([128, NS], BF16, tag=f"cb{e}")
                if e % 2 == 0:
                    nc.scalar.activation(cb[:], pcb[:], AF.Copy)
                else:
                    nc.vector.tensor_copy(cb[:], pcb[:])
                CB.append(cb)

            def cb4(e, k):
                return CB[e][:].unsqueeze(1).broadcast_to([128, k, NS])

            # W2 as raw f32 (shares the xx0f slot; converted on ACT later),
            # which shortens the gpsimd cast stream
            W2f = tp.tile([128, NJ2 * OUT], F32, tag="big")
            nc.sync.dma_start(W2f[:], w2_ext[:])
            nc.scalar.activation(W2[:], W2f[:], AF.Copy)

            # ---- layer 0 ----
            # chunk-major order matching weight-DMA arrival: all experts'
            # cond[0:128] chunks (w0a) first, then cond[128:] (w0b), then
            # latent+bias (w0c)
            XSLAT = []
            XC0 = []
            H1 = hp.tile([128, 4 * NS], BF16)
            ps_l0 = [psp.tile([128, NS], F32, name=f"psl0_{og}", tag=f"ps{og}")
                     for og in range(4)]
            for e in range(E):
                xc = xc0p.tile([128, 2 * NS], BF16, name=f"xc0_{e}", tag=f"xc0_{e}")
                nc.vector.tensor_mul(
                    xc[:].rearrange("p (c s) -> p c s", c=2),
                    X0[:, 0:2 * NS].rearrange("p (c s) -> p c s", c=2),
                    cb4(e, 2))
                XC0.append(xc)
            for e in range(E):
                t = latp.tile([128, NS], BF16, name=f"lat{e}", tag=f"lat{e}")
                nc.vector.tensor_mul(t[:], X0[:, 2 * NS:3 * NS], CB[e][:])
                XSLAT.append(t)
                if e == 0:
                    # bias rows: coeff^T into spare zero rows 64..71 of
                    # expert-0's latent chunk (w0h/w1h carry b0/b1 there)
                    nc.sync.dma_start(XSLAT[0][64:64 + E, :], CT[:])
            nfirst = True
            for c in range(L0C):
                for e in range(E):
                    j = e * L0C + c
                    rhs = (XC0[e][:, c * NS:(c + 1) * NS] if c < 2
                           else XSLAT[e][:])
                    for og in range(4):
                        nc.tensor.matmul(
                            ps_l0[og][:],
                            W0[:, j * H + og * 128: j * H + og * 128 + 128],
                            rhs, start=nfirst,
                            stop=(c == L0C - 1 and e == E - 1))
                    nfirst = False
            for og in range(4):
                gate_elu(ps_l0[og][:], H1[:, og * NS:(og + 1) * NS])

            # ---- layer 1 ----
            # latent chunks first: their inputs exist since layer 0, so the
            # PE crosses the L0->L1 boundary with zero stall while the L0
            # ELUs and the first L1 scale-TTs run on ACT/DVE.
            H2 = hp.tile([128, 4 * NS], BF16)
            ps_l1 = [psp.tile([128, NS], F32, name=f"psl1_{og}", tag=f"ps{og}")
                     for og in range(4)]
            for e in range(E):
                j = e * L1C + 4
                for og in range(4):
                    nc.tensor.matmul(
                        ps_l1[og][:],
                        W1[:, j * H + og * 128: j * H + og * 128 + 128],
                        XSLAT[e][:], start=(e == 0), stop=False)
            for e in range(E):
                xc = xsp.tile([128, 4 * NS], BF16, tag="xs")
                nc.vector.tensor_mul(
                    xc[:].rearrange("p (c s) -> p c s", c=4),
                    H1[:].rearrange("p (c s) -> p c s", c=4),
                    cb4(e, 4))
                if e < E - 1:
                    for c in range(4):
                        j = e * L1C + c
                        for og in range(4):
                            nc.tensor.matmul(
                                ps_l1[og][:],
                                W1[:, j * H + og * 128: j * H + og * 128 + 128],
                                xc[:, c * NS:(c + 1) * NS],
                                start=False, stop=False)
                else:
                    # last expert og-major so og0 finishes early and its ELU
                    # + L2 scale-TT overlap the remaining og matmuls
                    for og in range(4):
                        for c in range(4):
                            j = e * L1C + c
                            nc.tensor.matmul(
                                ps_l1[og][:],
                                W1[:, j * H + og * 128: j * H + og * 128 + 128],
                                xc[:, c * NS:(c + 1) * NS],
                                start=False, stop=(c == 3))
                        gate_elu(ps_l1[og][:], H2[:, og * NS:(og + 1) * NS])

            # ---- layer 2 (linear) ----
            ogs2 = ((0, 128), (128, OUT - 128))
            ps_l2 = [psp.tile([128, NS], F32, name=f"psl2_{og}", tag=f"ps{og}")
                     for og in range(2)]
            xcs2 = []
            for e in range(E):
                xc = xsp.tile([128, 4 * NS], BF16, tag="xs")
                nc.vector.tensor_mul(
                    xc[:].rearrange("p (c s) -> p c s", c=4),
                    H2[:].rearrange("p (c s) -> p c s", c=4),
                    cb4(e, 4))
                xcs2.append(xc)
                if e == E - 1:
                    break
                for c in range(L2C):
                    j = e * L2C + c
                    for og, (m0, msz) in enumerate(ogs2):
                        nc.tensor.matmul(
                            ps_l2[og][0:msz, :],
                            W2[:, j * OUT + m0: j * OUT + m0 + msz],
                            xc[:, c * NS:(c + 1) * NS],
                            start=(j == 0), stop=False)
            for og, (m0, msz) in enumerate(ogs2):
                e = E - 1
                for c in range(L2C):
                    j = e * L2C + c
                    nc.tensor.matmul(
                        ps_l2[og][0:msz, :],
                        W2[:, j * OUT + m0: j * OUT + m0 + msz],
                        xcs2[e][:, c * NS:(c + 1) * NS],
                        start=False, stop=False)
                nc.tensor.matmul(ps_l2[og][0:msz, :], B2[:, m0:m0 + msz], CT[:],
                                 start=False, stop=True)
                po = pop.tile([128, NS], F32, name=f"po{og}", tag="po")
                nc.scalar.activation(po[0:msz, :], ps_l2[og][0:msz, :], AF.Copy)
                nc.sync.dma_start(predt_ext[m0:m0 + msz, :], po[0:msz, :])

    nc.compile()
    return nc


def _prep(inputs):
    """Host-side shard/layout prep -> list of per-core input dicts."""
    lat = np.ascontiguousarray(np.asarray(inputs["latent"], dtype=np.float32))
    cond = np.ascontiguousarray(np.asarray(inputs["condition"], dtype=np.float32))
    ph = np.asarray(inputs["phase"], dtype=np.float32).reshape(N, PH2)
    gw1 = np.asarray(inputs["gate_w1"], dtype=np.float32)
    gw2 = np.asarray(inputs["gate_w2"], dtype=np.float32)
    gw3 = np.asarray(inputs["gate_w3"], dtype=np.float32)
    gb1 = np.asarray(inputs["gate_b1"], dtype=np.float32)
    gb2 = np.asarray(inputs["gate_b2"], dtype=np.float32)
    gb3 = np.asarray(inputs["gate_b3"], dtype=np.float32)
    w0 = np.asarray(inputs["w0"], dtype=np.float32)
    b0 = np.asarray(inputs["b0"], dtype=np.float32)
    w1 = np.asarray(inputs["w1"], dtype=np.float32)
    b1 = np.asarray(inputs["b1"], dtype=np.float32)
    w2 = np.asarray(inputs["w2"], dtype=np.float32)
    b2 = np.asarray(inputs["b2"], dtype=np.float32)

    xg_t = np.ascontiguousarray(np.concatenate([ph, lat], axis=1).T)  # [104, N]
    x0h = np.zeros((128, L0C, N), dtype=np.float32)
    x0h[:, 0, :] = cond.T[0:128]
    x0h[0:COND - 128, 1, :] = cond.T[128:COND]
    x0h[0:LAT, 2, :] = lat.T

    w0h = np.zeros((128, NJ0, H), dtype=np.float32)
    for e in range(E):
        w0h[:, e * L0C + 0] = w0[e, 0:128]
        w0h[0:COND - 128, e * L0C + 1] = w0[e, 128:COND]
        w0h[0:LAT, e * L0C + 2] = w0[e, COND:X0R]
    w0h[64:64 + E, 2] = b0  # bias rows ride expert-0's latent chunk
    w1h = np.zeros((128, NJ1, H), dtype=np.float32)
    for e in range(E):
        for c in range(4):
            w1h[:, e * L1C + c] = w1[e, c * 128:(c + 1) * 128]
        w1h[0:LAT, e * L1C + 4] = w1[e, H:H + LAT]
    w1h[64:64 + E, 4] = b1
    w2h = np.zeros((128, NJ2, OUT), dtype=np.float32)
    for e in range(E):
        for c in range(L2C):
            w2h[:, e * L2C + c] = w2[e, c * 128:(c + 1) * 128]

    # packed smalls: gw1|gw2|gw3|gb12|gb3|ident|b2
    smalls = np.zeros((128, 128 + 128 + E + 2 + 1 + 128 + OUT), dtype=np.float32)
    smalls[0:GIN, 0:128] = gw1
    smalls[:, 128:256] = gw2
    smalls[:, 256:256 + E] = gw3
    smalls[:, 264] = gb1
    smalls[:, 265] = gb2
    smalls[0:E, 266] = gb3
    smalls[:, 267:395] = np.eye(128, dtype=np.float32)
    smalls[0:E, 395:395 + OUT] = b2

    xgx0 = np.zeros((128, 1 + L0C, N), dtype=np.float32)
    xgx0[0:GIN, 0, :] = xg_t
    xgx0[:, 1:, :] = x0h

    w0v = w0h.reshape(128, E, L0C, H)
    w1v = w1h.reshape(128, E, L1C, H)
    shared = {
        "smalls": smalls,
        "w0a": np.ascontiguousarray(w0v[:, :, 0, :]),
        "w0b": np.ascontiguousarray(w0v[:, :, 1, :]),
        "w0c": np.ascontiguousarray(w0v[:, :, 2, :]),
        "w1l": np.ascontiguousarray(w1v[:, :, 4, :]),
        "w1a": np.ascontiguousarray(w1v[:, 0:4, 0:4, :]),
        "w1b": np.ascontiguousarray(w1v[:, 4:8, 0:4, :]),
        "w2": w2h,
    }
    in_maps = []
    for ci in range(NCORES):
        s = slice(ci * NS, (ci + 1) * NS)
        m = {"xgx0": np.ascontiguousarray(xgx0[:, :, s])}
        m.update(shared)
        in_maps.append(m)
    return in_maps


def kernel(**inputs):
    if "nc" not in _CACHE:
        _CACHE["nc"] = build()
    nc = _CACHE["nc"]
    in_maps = _prep(inputs)
    res = run_bass_kernel_spmd(nc, in_maps, core_ids=list(range(NCORES)))
    preds, coeffs = [], []
    for ci in range(NCORES):
        r = res.results[ci]
        preds.append(np.ascontiguousarray(r["predt"].T))          # [NS, OUT]
        c4 = r["coeff4"]                                          # [128, 4, E]
        coeffs.append(np.ascontiguousarray(
            c4.transpose(1, 0, 2).reshape(NS, E)))
    pred = np.concatenate(preds, axis=0).astype(np.float32)
    coeff = np.concatenate(coeffs, axis=0).astype(np.float32)
    return pred, coeff


# revision 25
# speedup vs baseline: 1.2328x; 1.0035x over previous
"""Trainium2 Bass kernel for nn_Application_85469849191065 (moe_routing).

Data-parallel over the batch dim N=4096 across 8 NeuronCores (512 samples
per core); the small expert weight banks are replicated to every core.

On-device layout is feature-major (activations stored transposed,
[features, samples]) so every matmul's contraction dim lands on the
partition axis with no per-layer transposes.  The per-sample expert blend
  out_n = sum_e coeff[n,e] * (x_n @ W[e] + b[e])
is computed as scale-input MoE: the moving operand of each contraction
chunk is (coeff_e broadcast across partitions) * X^T, so the blend over
experts accumulates for free in PSUM.  The bias term rides spare zero
rows of the latent contraction chunk whose moving rows hold coeff^T.
Compute dtype is bf16 (weights cast f32->bf16 in-flight by the gpsimd
DMA engines), accumulation fp32 in PSUM.
"""

import numpy as np

import concourse.bass as bass
import concourse.bacc as bacc
import concourse.mybir as mybir
import concourse.tile as tile
from concourse.bass_utils import run_bass_kernel_spmd

F32 = mybir.dt.float32
BF16 = mybir.dt.bfloat16
AF = mybir.ActivationFunctionType
ALU = mybir.AluOpType

N = 4096
NCORES = 8
NS = N // NCORES  # 512 samples per core
LAT, COND, PH2, E = 64, 219, 40, 8
H, OUT = 512, 171
GIN = PH2 + LAT  # 104
X0R = COND + LAT  # 283

L0C = 3   # per-expert L0 chunks: cond[0:128], cond[128:219], latent(64)
L1C = 5   # 4 h chunks + the shared latent chunk
L2C = 4
NJ0 = E * L0C  # 24
NJ1 = E * L1C  # 40
NJ2 = E * L2C  # 32

_CACHE = {}


def build():
    nc = bacc.Bacc("TRN2", target_bir_lowering=False, debug=False,
                   num_devices=NCORES)

    # ---- dram parameters (per-core shard shapes) ----
    # xgx0: chunk0 = gate input (104 rows), chunks 1..3 = x0 chunks
    xgx0_ext = nc.dram_tensor("xgx0", [128, 1 + L0C, NS], F32,
                              kind="ExternalInput")
    # smalls: gw1|gw2|gw3|gb12|gb3|ident|b2  packed along free dim
    smalls_ext = nc.dram_tensor("smalls", [128, 128 + 128 + E + 2 + 1 + 128 + OUT],
                                F32, kind="ExternalInput")
    # trimmed weight groups (no zero-pad rows on the wire)
    w0a_ext = nc.dram_tensor("w0a", [128, E, H], F32, kind="ExternalInput")
    w0b_ext = nc.dram_tensor("w0b", [128, E, H], F32, kind="ExternalInput")
    w0c_ext = nc.dram_tensor("w0c", [128, E, H], F32, kind="ExternalInput")
    w1l_ext = nc.dram_tensor("w1l", [128, E, H], F32, kind="ExternalInput")
    w1a_ext = nc.dram_tensor("w1a", [128, 4, 4, H], F32, kind="ExternalInput")
    w1b_ext = nc.dram_tensor("w1b", [128, 4, 4, H], F32, kind="ExternalInput")
    w2_ext = nc.dram_tensor("w2", [128, NJ2, OUT], F32, kind="ExternalInput")
    predt_ext = nc.dram_tensor("predt", [OUT, NS], F32, kind="ExternalOutput")
    coeff4_ext = nc.dram_tensor("coeff4", [128, NS // 128, E], F32,
                                kind="ExternalOutput")

    with tile.TileContext(nc) as tc:
        with (
            tc.tile_pool(name="w", bufs=1) as wp,
            tc.tile_pool(name="tmp", bufs=1) as tp,
            tc.tile_pool(name="xs", bufs=4) as xsp,
            tc.tile_pool(name="xc0", bufs=1) as xc0p,
            tc.tile_pool(name="lat", bufs=1) as latp,
            tc.tile_pool(name="h", bufs=1) as hp,
            tc.tile_pool(name="elu", bufs=2) as ep,
            tc.tile_pool(name="cb", bufs=1) as cbp,
            tc.tile_pool(name="sm", bufs=1) as smp,
            tc.tile_pool(name="po", bufs=2) as pop,
            tc.tile_pool(name="ps", bufs=1, space="PSUM") as psp,
            tc.tile_pool(name="psg", bufs=2, space="PSUM") as psg,
            tc.tile_pool(name="pss", bufs=2, space="PSUM") as pss,
        ):
            # ---- persistent weight/activation tiles ----
            SM = wp.tile([128, 128 + 128 + E + 2 + 1 + 128 + OUT], F32)
            XX0f = tp.tile([128, (1 + L0C) * NS], F32, tag="big")
            W0 = wp.tile([128, NJ0 * H], BF16)
            W1 = wp.tile([128, NJ1 * H], BF16)
            W2 = wp.tile([128, NJ2 * OUT], BF16)

            # latency-critical small loads on the sync engine: its HWDGE
            # issues start ~4us before the gpsimd preamble finishes, and the
            # DMA engines are still empty so they land fast
            nc.sync.dma_start(SM[:], smalls_ext[:])
            nc.sync.dma_start(XX0f[:], xgx0_ext[:])
            # weight casts f32->bf16 on gpsimd SWDGE, zero-pad rows skipped
            W0v = W0[:].rearrange("p (j h) -> p j h", j=NJ0)
            W1v = W1[:].rearrange("p (j h) -> p j h", j=NJ1)
            half = 4 * L0C
            nc.gpsimd.dma_start(W0v[:, 0:half:L0C, :], w0a_ext[:, 0:4, :])
            nc.gpsimd.dma_start(W0v[:, half:NJ0:L0C, :], w0a_ext[:, 4:8, :])
            nc.gpsimd.dma_start(W0v[:, 1:half:L0C, :], w0b_ext[:, 0:4, :])
            nc.gpsimd.dma_start(W0v[:, half + 1:NJ0:L0C, :], w0b_ext[:, 4:8, :])
            nc.gpsimd.dma_start(W0v[:, 2:half:L0C, :], w0c_ext[:, 0:4, :])
            nc.gpsimd.dma_start(W0v[:, half + 2:NJ0:L0C, :], w0c_ext[:, 4:8, :])
            nc.gpsimd.dma_start(W1v[:, 4:NJ1:L1C, :], w1l_ext[:])
            W1h = W1[:].rearrange("p (e c h) -> p e c h", e=E, c=L1C)
            nc.gpsimd.dma_start(W1h[:, 0:4, 0:4, :], w1a_ext[:])
            nc.gpsimd.dma_start(W1h[:, 4:8, 0:4, :], w1b_ext[:])

            # slice views into the packed smalls tile
            GW1f = SM[0:GIN, 0:128]
            GW2f = SM[:, 128:256]
            GW3f = SM[:, 256:256 + E]
            GB12 = SM[:, 264:266]
            GB3 = SM[0:E, 266:267]
            IDT = SM[:, 267:395]
            B2f = SM[0:E, 395:395 + OUT]

            # bf16 converts (DVE for the small gate weights, ACT for inputs)
            GWB = wp.tile([128, 256 + E], BF16)
            GW1 = GWB[0:GIN, 0:128]
            GW2 = GWB[:, 128:256]
            GW3 = GWB[:, 256:256 + E]
            B2 = wp.tile([E, OUT], BF16)
            XX0 = wp.tile([128, (1 + L0C) * NS], BF16)
            XG = XX0[0:GIN, 0:NS]
            X0 = XX0[:, NS:(1 + L0C) * NS]
            nc.vector.tensor_copy(GWB[:], SM[:, 0:256 + E])
            nc.vector.tensor_copy(B2[:], B2f)
            nc.scalar.activation(XX0[:, 0:NS], XX0f[:, 0:NS], AF.Copy)
            nc.scalar.activation(XX0[:, NS:(1 + L0C) * NS],
                                 XX0f[:, NS:(1 + L0C) * NS], AF.Copy)

            # ---- gate network (feature-major, bf16 matmuls) ----
            def gate_elu(ps_in, out_tile, bias=0.0):
                e = ep.tile([128, NS], BF16, tag="elu_e")
                r = ep.tile([128, NS], BF16, tag="elu_r")
                nc.scalar.activation(e[:], ps_in, AF.Exp, bias=bias)
                nc.scalar.activation(r[:], ps_in, AF.Relu, bias=bias)
                nc.vector.scalar_tensor_tensor(out_tile, e[:], -1.0, r[:],
                                               ALU.add, ALU.min)

            pg1 = psg.tile([128, NS], F32, tag="pg")
            nc.tensor.matmul(pg1[:], GW1, XG, start=True, stop=True)
            G1 = tp.tile([128, NS], BF16, tag="g1")
            gate_elu(pg1[:], G1[:], GB12[:, 0:1])

            pg2 = psg.tile([128, NS], F32, tag="pg")
            nc.tensor.matmul(pg2[:], GW2, G1[:], start=True, stop=True)
            G2 = tp.tile([128, NS], BF16, tag="g2")
            gate_elu(pg2[:], G2[:], GB12[:, 1:2])

            pgl = pss.tile([E, NS], F32, tag="sm")
            nc.tensor.matmul(pgl[:], GW3, G2[:], start=True, stop=True)
            EXPF = smp.tile([E, NS], F32)
            nc.scalar.activation(EXPF[:], pgl[:], AF.Exp, bias=GB3)

            # softmax normalization, sample-major
            E4 = smp.tile([128, NS // 128, E], F32)
            for c in range(NS // 128):
                pt = pss.tile([128, E], F32, tag="sm")
                nc.tensor.transpose(pt[:], EXPF[:, c * 128:(c + 1) * 128],
                                    IDT[0:E, 0:E])
                nc.vector.tensor_copy(E4[:, c, :], pt[:])
            S4 = smp.tile([128, NS // 128], F32)
            nc.vector.tensor_reduce(S4[:], E4[:], mybir.AxisListType.X, ALU.add)
            R4 = smp.tile([128, NS // 128], F32)
            nc.vector.reciprocal(R4[:], S4[:])
            C4 = smp.tile([128, NS // 128, E], F32)
            for c in range(NS // 128):
                nc.vector.tensor_scalar_mul(C4[:, c, :], E4[:, c, :],
                                            R4[:, c:c + 1])
            nc.sync.dma_start(coeff4_ext[:], C4[:])

            # coeff^T [E, NS] bf16 (bias rows + L2 bias matmuls)
            CT = smp.tile([E, NS], BF16)
            for c in range(NS // 128):
                ptc = pss.tile([E, 128], F32, tag="sm")
                nc.tensor.transpose(ptc[:], C4[:, c, :], IDT[:, 0:128])
                nc.vector.tensor_copy(CT[:, c * 128:(c + 1) * 128], ptc[:])
            # flatten rows into partition 0, then DMA-broadcast each row to
            # all 128 partitions (no compute engine involved)
            CTS = smp.tile([1, E * NS], BF16)
            nc.sync.dma_start(CTS[:], CT[:])
            ONES = smp.tile([1, 128], BF16)
            nc.vector.memset(ONES[:], 1.0)
            CB = []
            for e in range(E):
                pcb = psg.tile([128, NS], F32, name=f"pcb{e}", tag="pg")
                nc.tensor.matmul(pcb[:], ONES[:],
                                 CTS[0:1, e * NS:(e + 1) * NS],
                                 start=True, stop=True)
                cb = cbp.tile([128, NS], BF16, tag=f"cb{e}")
                nc.scalar.activation(cb[:], pcb[:], AF.Copy)
                CB.append(cb)

            def cb4(e, k):
                return CB[e][:].unsqueeze(1).broadcast_to([128, k, NS])

            # W2 as raw f32 (shares the xx0f slot; converted on ACT later),
            # which shortens the gpsimd cast stream
            W2f = tp.tile([128, NJ2 * OUT], F32, tag="big")
            nc.sync.dma_start(W2f[:], w2_ext[:])
            nc.scalar.activation(W2[:], W2f[:], AF.Copy)

            # ---- layer 0 ----
            # chunk-major order matching weight-DMA arrival: all experts'
            # cond[0:128] chunks (w0a) first, then cond[128:] (w0b), then
            # latent+bias (w0c)
            XSLAT = []
            XC0 = []
            H1 = hp.tile([128, 4 * NS], BF16)
            ps_l0 = [psp.tile([128, NS], F32, name=f"psl0_{og}", tag=f"ps{og}")
                     for og in range(4)]
            for e in range(E):
                xc = xc0p.tile([128, 2 * NS], BF16, name=f"xc0_{e}", tag=f"xc0_{e}")
                nc.vector.tensor_mul(
                    xc[:].rearrange("p (c s) -> p c s", c=2),
                    X0[:, 0:2 * NS].rearrange("p (c s) -> p c s", c=2),
                    cb4(e, 2))
                XC0.append(xc)
            for e in range(E):
                t = latp.tile([128, NS], BF16, name=f"lat{e}", tag=f"lat{e}")
                nc.vector.tensor_mul(t[:], X0[:, 2 * NS:3 * NS], CB[e][:])
                XSLAT.append(t)
                if e == 0:
                    # bias rows: coeff^T into spare zero rows 64..71 of
                    # expert-0's latent chunk (w0h/w1h carry b0/b1 there)
                    nc.sync.dma_start(XSLAT[0][64:64 + E, :], CT[:])
            nfirst = True
            for c in range(L0C):
                for e in range(E):
                    j = e * L0C + c
                    rhs = (XC0[e][:, c * NS:(c + 1) * NS] if c < 2
                           else XSLAT[e][:])
                    for og in range(4):
                        nc.tensor.matmul(
                            ps_l0[og][:],
                            W0[:, j * H + og * 128: j * H + og * 128 + 128],
                            rhs, start=nfirst,
                            stop=(c == L0C - 1 and e == E - 1))
                    nfirst = False
            for og in range(4):
                gate_elu(ps_l0[og][:], H1[:, og * NS:(og + 1) * NS])

            # ---- layer 1 ----
            # latent chunks first: their inputs exist since layer 0, so the
            # PE crosses the L0->L1 boundary with zero stall while the L0
            # ELUs and the first L1 scale-TTs run on ACT/DVE.
            H2 = hp.tile([128, 4 * NS], BF16)
            ps_l1 = [psp.tile([128, NS], F32, name=f"psl1_{og}", tag=f"ps{og}")
                     for og in range(4)]
            for e in range(E):
                j = e * L1C + 4
                for og in range(4):
                    nc.tensor.matmul(
                        ps_l1[og][:],
                        W1[:, j * H + og * 128: j * H + og * 128 + 128],
                        XSLAT[e][:], start=(e == 0), stop=False)
            for e in range(E):
                xc = xsp.tile([128, 4 * NS], BF16, tag="xs")
                nc.vector.tensor_mul(
                    xc[:].rearrange("p (c s) -> p c s", c=4),
                    H1[:].rearrange("p (c s) -> p c s", c=4),
                    cb4(e, 4))
                if e < E - 1:
                    for c in range(4):
                        j = e * L1C + c
                        for og in range(4):
                            nc.tensor.matmul(
                                ps_l1[og][:],
                                W1[:, j * H + og * 128: j * H + og * 128 + 128],
                                xc[:, c * NS:(c + 1) * NS],
                                start=False, stop=False)
                else:
                    # last expert og-major so og0 finishes early and its ELU
                    # + L2 scale-TT overlap the remaining og matmuls
                    for og in range(4):
                        for c in range(4):
                            j = e * L1C + c
                            nc.tensor.matmul(
                                ps_l1[og][:],
                                W1[:, j * H + og * 128: j * H + og * 128 + 128],
                                xc[:, c * NS:(c + 1) * NS],
                                start=False, stop=(c == 3))
                        gate_elu(ps_l1[og][:], H2[:, og * NS:(og + 1) * NS])

            # ---- layer 2 (linear) ----
            ogs2 = ((0, 128), (128, OUT - 128))
            ps_l2 = [psp.tile([128, NS], F32, name=f"psl2_{og}", tag=f"ps{og}")
                     for og in range(2)]
            xcs2 = []
            for e in range(E):
                xc = xsp.tile([128, 4 * NS], BF16, tag="xs")
                if e == 0:
                    # og-sliced so the first matmul only waits on H2[og0]
                    for c in range(4):
                        nc.vector.tensor_mul(
                            xc[:, c * NS:(c + 1) * NS],
                            H2[:, c * NS:(c + 1) * NS], CB[e][:])
                else:
                    nc.vector.tensor_mul(
                        xc[:].rearrange("p (c s) -> p c s", c=4),
                        H2[:].rearrange("p (c s) -> p c s", c=4),
                        cb4(e, 4))
                xcs2.append(xc)
                if e == E - 1:
                    break
                for c in range(L2C):
                    j = e * L2C + c
                    for og, (m0, msz) in enumerate(ogs2):
                        nc.tensor.matmul(
                            ps_l2[og][0:msz, :],
                            W2[:, j * OUT + m0: j * OUT + m0 + msz],
                            xc[:, c * NS:(c + 1) * NS],
                            start=(j == 0), stop=False)
            for og, (m0, msz) in enumerate(ogs2):
                e = E - 1
                for c in range(L2C):
                    j = e * L2C + c
                    nc.tensor.matmul(
                        ps_l2[og][0:msz, :],
                        W2[:, j * OUT + m0: j * OUT + m0 + msz],
                        xcs2[e][:, c * NS:(c + 1) * NS],
                        start=False, stop=False)
                nc.tensor.matmul(ps_l2[og][0:msz, :], B2[:, m0:m0 + msz], CT[:],
                                 start=False, stop=True)
                po = pop.tile([128, NS], F32, name=f"po{og}", tag="po")
                nc.scalar.activation(po[0:msz, :], ps_l2[og][0:msz, :], AF.Copy)
                nc.sync.dma_start(predt_ext[m0:m0 + msz, :], po[0:msz, :])

    nc.compile()
    return nc


def _prep(inputs):
    """Host-side shard/layout prep -> list of per-core input dicts."""
    lat = np.ascontiguousarray(np.asarray(inputs["latent"], dtype=np.float32))
    cond = np.ascontiguousarray(np.asarray(inputs["condition"], dtype=np.float32))
    ph = np.asarray(inputs["phase"], dtype=np.float32).reshape(N, PH2)
    gw1 = np.asarray(inputs["gate_w1"], dtype=np.float32)
    gw2 = np.asarray(inputs["gate_w2"], dtype=np.float32)
    gw3 = np.asarray(inputs["gate_w3"], dtype=np.float32)
    gb1 = np.asarray(inputs["gate_b1"], dtype=np.float32)
    gb2 = np.asarray(inputs["gate_b2"], dtype=np.float32)
    gb3 = np.asarray(inputs["gate_b3"], dtype=np.float32)
    w0 = np.asarray(inputs["w0"], dtype=np.float32)
    b0 = np.asarray(inputs["b0"], dtype=np.float32)
    w1 = np.asarray(inputs["w1"], dtype=np.float32)
    b1 = np.asarray(inputs["b1"], dtype=np.float32)
    w2 = np.asarray(inputs["w2"], dtype=np.float32)
    b2 = np.asarray(inputs["b2"], dtype=np.float32)

    xg_t = np.ascontiguousarray(np.concatenate([ph, lat], axis=1).T)  # [104, N]
    x0h = np.zeros((128, L0C, N), dtype=np.float32)
    x0h[:, 0, :] = cond.T[0:128]
    x0h[0:COND - 128, 1, :] = cond.T[128:COND]
    x0h[0:LAT, 2, :] = lat.T

    w0h = np.zeros((128, NJ0, H), dtype=np.float32)
    for e in range(E):
        w0h[:, e * L0C + 0] = w0[e, 0:128]
        w0h[0:COND - 128, e * L0C + 1] = w0[e, 128:COND]
        w0h[0:LAT, e * L0C + 2] = w0[e, COND:X0R]
    w0h[64:64 + E, 2] = b0  # bias rows ride expert-0's latent chunk
    w1h = np.zeros((128, NJ1, H), dtype=np.float32)
    for e in range(E):
        for c in range(4):
            w1h[:, e * L1C + c] = w1[e, c * 128:(c + 1) * 128]
        w1h[0:LAT, e * L1C + 4] = w1[e, H:H + LAT]
    w1h[64:64 + E, 4] = b1
    w2h = np.zeros((128, NJ2, OUT), dtype=np.float32)
    for e in range(E):
        for c in range(L2C):
            w2h[:, e * L2C + c] = w2[e, c * 128:(c + 1) * 128]

    # packed smalls: gw1|gw2|gw3|gb12|gb3|ident|b2
    smalls = np.zeros((128, 128 + 128 + E + 2 + 1 + 128 + OUT), dtype=np.float32)
    smalls[0:GIN, 0:128] = gw1
    smalls[:, 128:256] = gw2
    smalls[:, 256:256 + E] = gw3
    smalls[:, 264] = gb1
    smalls[:, 265] = gb2
    smalls[0:E, 266] = gb3
    smalls[:, 267:395] = np.eye(128, dtype=np.float32)
    smalls[0:E, 395:395 + OUT] = b2

    xgx0 = np.zeros((128, 1 + L0C, N), dtype=np.float32)
    xgx0[0:GIN, 0, :] = xg_t
    xgx0[:, 1:, :] = x0h

    w0v = w0h.reshape(128, E, L0C, H)
    w1v = w1h.reshape(128, E, L1C, H)
    shared = {
        "smalls": smalls,
        "w0a": np.ascontiguousarray(w0v[:, :, 0, :]),
        "w0b": np.ascontiguousarray(w0v[:, :, 1, :]),
        "w0c": np.ascontiguousarray(w0v[:, :, 2, :]),
        "w1l": np.ascontiguousarray(w1v[:, :, 4, :]),
        "w1a": np.ascontiguousarray(w1v[:, 0:4, 0:4, :]),
        "w1b": np.ascontiguousarray(w1v[:, 4:8, 0:4, :]),
        "w2": w2h,
    }
    in_maps = []
    for ci in range(NCORES):
        s = slice(ci * NS, (ci + 1) * NS)
        m = {"xgx0": np.ascontiguousarray(xgx0[:, :, s])}
        m.update(shared)
        in_maps.append(m)
    return in_maps


def kernel(**inputs):
    if "nc" not in _CACHE:
        _CACHE["nc"] = build()
    nc = _CACHE["nc"]
    in_maps = _prep(inputs)
    res = run_bass_kernel_spmd(nc, in_maps, core_ids=list(range(NCORES)))
    preds, coeffs = [], []
    for ci in range(NCORES):
        r = res.results[ci]
        preds.append(np.ascontiguousarray(r["predt"].T))          # [NS, OUT]
        c4 = r["coeff4"]                                          # [128, 4, E]
        coeffs.append(np.ascontiguousarray(
            c4.transpose(1, 0, 2).reshape(NS, E)))
    pred = np.concatenate(preds, axis=0).astype(np.float32)
    coeff = np.concatenate(coeffs, axis=0).astype(np.float32)
    return pred, coeff
